# revision 1
# baseline (speedup 1.0000x reference)
"""Trainium2 Bass kernel for nn_Decoder_14894946583396 (dense_mlp).

Reference computation:
    sized = broadcast(representation[B,1,R] -> [B,S,R])   (ones @ rep)
    h     = relu(sized @ W1^T + b1)                       [B,S,HID]
    out   = h @ W2^T + b2                                 [B,S,OUT]

Because every position s within batch b receives the identical input row
representation[b], the MLP output row is identical for all S positions:
    row[b] = relu(rep[b] @ W1^T + b1) @ W2^T + b2         [B,OUT]
    out[b, s, :] = row[b]  for all s

The kernel computes the tiny per-batch MLP on the TensorEngine (fp32,
bit-exact vs the f32 reference) and broadcast-writes each row across S
with wide SBUF->DRAM DMAs. Data-parallel across 8 NeuronCores: 4 batches
per core, replicated weights.

Device pipeline per core:
  1. Four input DMAs: pk1a = {x^T, I4} (tiny, HWDGE lane 0 — it gates
     compute), prow = {b1, ones, b2} single row, w1 = W1^T, w2 = W2^T
     (all three on SWDGE lanes, streaming under the warmup).
  2. ~10 us of dummy matmuls on zeros warm the PE HAM clock gate
     (1.2 -> 2.4 GHz) while weights stream in.
  3. L1: H[m,h] = x @ W1^T via 8 accumulating matmuls with the tiny x^T
     chunk as stationary (cheap LDWEIGHTS), bias folded in as a K=1
     ones-matmul, relu on ScalarE.
  4. H -> H^T via 4 PE transposes (stationary operand for L2).
  5. L2: Y[m,o] = H @ W2^T + b2, 10 matmuls into 2 PSUM banks.
  6. Y rows moved to partition-0 tiles by tiny SBUF->SBUF DMAs (matmul
     operands must start at partition 0/32/64).
  7. Broadcast: K=1 matmul with a ones row as stationary -> [128,512]
     PSUM tiles where every partition holds row[b]; one PSUM->SBUF copy
     per half, then wide SBUF->SBUF replication copies (one writer
     engine per tile).
  8. 8 output DMAs of 2 MiB each on the 8 fresh HWDGE lanes.

Single-sync-wait discipline (this walrus rejects 2+ waits on any
instruction): inputs are packed so every consumer sees one DMA
semaphore; SWDGE lanes carry everything but pk1a and the outputs so no
HWDGE output trigger reuses a lane whose data wait is unobserved;
artificial add_dep_helper edges pre-observe upcoming DMA lanes on
instructions that have a free wait slot; and a chain of 1-wait SP nops
before the TileContext exit drain leaves the drain with nothing to wait
on.
"""

import sys

import numpy as np

if "/opt/trn_rl_repo" not in sys.path:
    sys.path.insert(0, "/opt/trn_rl_repo")

B, S, R = 32, 1024, 1024
HID, OUT = 512, 1024
N_CORES = 8
BPC = B // N_CORES  # batches per core

RC = R // 128  # layer-1 contraction chunks
HC = HID // 128  # layer-2 contraction chunks
OC = OUT // 512  # 512-wide output column chunks

# pk1a columns: [p, rc*BPC + m] = rep[m, rc*128+p], then a 4x4 identity,
# then 4 selector-broadcast blocks: [k, SELOFF + b*128 + m] = (k == b)
XTOFF = 0
I4OFF = XTOFF + RC * BPC
SELOFF = I4OFF + BPC
PK1AW = SELOFF + BPC * 128
# prow columns (single partition row)
B1OFF = 0
ONOFF = B1OFF + HID
B2OFF = ONOFF + 128
PROWW = B2OFF + OUT
# w1: [p, rc*HID + h] = W1[h, rc*128+p];  w2: [p, hc*OUT + o] = W2[o, hc*128+p]

N_COPIES = 4  # row copies along the free dim of each broadcast tile
S_PER_DMA = 128 * N_COPIES  # s-positions covered per output DMA
N_DMAS = S // S_PER_DMA  # output DMAs per batch
N_WARMUP = 8

_CACHED_NC = None


def _build_nc():
    import concourse.bass as bass
    import concourse.mybir as mybir
    from concourse.tile import TileContext, add_dep_helper

    f32 = mybir.dt.float32
    relu = mybir.ActivationFunctionType.Relu
    fcopy = mybir.ActivationFunctionType.Copy
    nc = bass.Bass()

    pk1a = nc.dram_tensor("pk1a", [128, PK1AW], f32, kind="ExternalInput")
    prow = nc.dram_tensor("prow", [1, PROWW], f32, kind="ExternalInput")
    w1 = nc.dram_tensor("w1", [128, RC * HID], f32, kind="ExternalInput")
    w2 = nc.dram_tensor("w2", [128, HC * OUT], f32, kind="ExternalInput")
    out = nc.dram_tensor("out", [BPC, S, OUT], f32, kind="ExternalOutput")

    with TileContext(nc) as tc:
        with (
            tc.tile_pool(name="const", bufs=1) as cpool,
            tc.tile_pool(name="psum_s", bufs=1, space="PSUM") as pp_s,
            tc.tile_pool(name="psum_y", bufs=2, space="PSUM") as pp_y,
            tc.tile_pool(name="psum_t", bufs=1, space="PSUM") as pp_t,
            tc.tile_pool(name="psum_bc", bufs=4, space="PSUM") as pp_bc,
        ):
            p1a = cpool.tile([128, PK1AW], f32, tag="pk1a")
            nc.sync.dma_start(out=p1a[:, :], in_=pk1a[:, :])
            prow_sb = cpool.tile([1, PROWW], f32, tag="prow")
            dma_prow = nc.gpsimd.dma_start(out=prow_sb[0:1, :], in_=prow[0:1, :])
            w1_sb = cpool.tile([128, RC * HID], f32, tag="w1")
            dma_w1 = nc.gpsimd.dma_start(out=w1_sb[:, :], in_=w1[:, :])
            w2_sb = cpool.tile([128, HC * OUT], f32, tag="w2")
            dma_w2 = nc.gpsimd.dma_start(out=w2_sb[:, :], in_=w2[:, :])

            # ---- PE warmup on zeros; shares L1's PSUM tile (a slot handoff
            # would emit a non-elidable same-engine wait) -------------------
            wm_sb = cpool.tile([128, 512], f32, tag="wm")
            nc.vector.memset(wm_sb[:, :], 0.0)
            ph_full = pp_s.tile([128, HID], f32, tag="s")
            for k in range(N_WARMUP):
                wmm = nc.tensor.matmul(
                    ph_full[:, :],
                    lhsT=wm_sb[:, 0:128],
                    rhs=wm_sb[:, :],
                    start=True,
                    stop=True,
                )
            # the last warmup matmul observes w1's lane so L1's first matmul
            # only needs the pk1a wait
            add_dep_helper(wmm.ins, dma_w1.ins, sync=True, reason="observe w1")

            # ---- L1: H[m, h] = x @ W1^T + b1, relu -------------------------
            ph = ph_full[0:BPC, :]
            for rc in range(RC):
                mm = nc.tensor.matmul(
                    ph[:, :],
                    lhsT=p1a[:, XTOFF + rc * BPC : XTOFF + (rc + 1) * BPC],
                    rhs=w1_sb[:, rc * HID : rc * HID + HID],
                    start=(rc == 0),
                    stop=False,
                )
            # rc=7 has a free wait slot: pre-observe w2's lane for L2
            add_dep_helper(mm.ins, dma_w2.ins, sync=True, reason="observe w2")
            nc.tensor.matmul(
                ph[:, :],
                lhsT=prow_sb[0:1, ONOFF : ONOFF + BPC],
                rhs=prow_sb[0:1, B1OFF : B1OFF + HID],
                start=False,
                stop=True,
            )
            h_sb = cpool.tile([BPC, HID], f32, tag="h")
            nc.scalar.activation(h_sb[:, :], ph[:, :], relu)

            # ---- H -> H^T (stationary operand for L2) ----------------------
            ht_sb = cpool.tile([128, HC * BPC], f32, tag="ht")
            for hc in range(HC):
                pt = pp_t.tile([128, BPC], f32, tag="t")
                nc.tensor.transpose(
                    pt[:, :],
                    h_sb[0:BPC, hc * 128 : (hc + 1) * 128],
                    p1a[0:BPC, I4OFF : I4OFF + BPC],
                )
                nc.scalar.activation(
                    ht_sb[:, hc * BPC : (hc + 1) * BPC], pt[:, :], fcopy
                )

            # ---- L2: Y[m, o] = H @ W2^T + b2 -------------------------------
            # per-oc Y tiles so the broadcast of the first half can start
            # while the second half's matmuls still run
            y_halves = []
            for oc in range(OC):
                py = pp_y.tile([BPC, 512], f32, tag="y")
                for hc in range(HC):
                    nc.tensor.matmul(
                        py[:, :],
                        lhsT=ht_sb[:, hc * BPC : (hc + 1) * BPC],
                        rhs=w2_sb[:, hc * OUT + oc * 512 : hc * OUT + oc * 512 + 512],
                        start=(hc == 0),
                        stop=False,
                    )
                nc.tensor.matmul(
                    py[:, :],
                    lhsT=prow_sb[0:1, ONOFF : ONOFF + BPC],
                    rhs=prow_sb[0:1, B2OFF + oc * 512 : B2OFF + (oc + 1) * 512],
                    start=False,
                    stop=True,
                )
                yh = cpool.tile([BPC, 512], f32, tag=f"yh{oc}")
                nc.vector.tensor_copy(yh[:, :], py[:, :])
                y_halves.append(yh)

            # ---- broadcast rows across partitions, replicate, store --------
            # A K=4 selector matmul (lhsT = e_b outer ones, host-packed)
            # extracts row b of Y AND replicates it across all 128 output
            # partitions in one PE op — both operands at base partition 0.
            out_dmas = []
            for b in range(BPC):
                yt = cpool.tile([128, N_COPIES * OUT], f32, tag=f"yt{b}")
                copy_eng = "dve" if b % 2 == 0 else "act"
                for oc in range(OC):
                    pb = pp_bc.tile([128, 512], f32, tag="bc")
                    mm = nc.tensor.matmul(
                        pb[:, :],
                        lhsT=p1a[0:BPC, SELOFF + b * 128 : SELOFF + (b + 1) * 128],
                        rhs=y_halves[oc][0:BPC, :],
                        start=True,
                        stop=True,
                    )
                    last_mm = mm
                    # PSUM -> SBUF once per oc half...
                    dst = yt[:, oc * 512 : (oc + 1) * 512]
                    if copy_eng == "dve":
                        last_dve = nc.vector.tensor_copy(dst, pb[:, :])
                    else:
                        last_act = nc.scalar.activation(dst, pb[:, :], fcopy)
                # ...then replicate with wide SBUF->SBUF copies (2x f32 mode)
                for c in range(1, N_COPIES):
                    dst = yt[:, c * OUT : (c + 1) * OUT]
                    if copy_eng == "dve":
                        last_dve = nc.vector.tensor_copy(dst, yt[:, 0:OUT])
                    else:
                        last_act = nc.scalar.activation(dst, yt[:, 0:OUT], fcopy)
                # each DMA writes S_PER_DMA consecutive s rows (all identical)
                for j in range(N_DMAS):
                    d = nc.sync.dma_start(
                        out=out[b, j * S_PER_DMA : (j + 1) * S_PER_DMA, :].rearrange(
                            "(p c) o -> p c o", c=N_COPIES
                        ),
                        in_=yt[:, :].rearrange("p (c o) -> p c o", o=OUT),
                    )
                    out_dmas.append(d)

            # The kernel-tail drain waits on every proc's final tick, but this
            # walrus allows at most ONE sync wait per instruction. Chain SP
            # nops, one dependency each, so SP's vector clock observes the
            # final tick of every DMA lane and engine before the drain.
            tail = out_dmas + [dma_prow, dma_w1, dma_w2, last_mm, last_act, last_dve]
            for d in tail:
                n = nc.sync.nop(nofuse=True)
                add_dep_helper(
                    n.ins, d.ins, sync=True, reason="observe final ticks pre-drain"
                )

    return nc


def _get_nc():
    global _CACHED_NC
    if _CACHED_NC is None:
        _CACHED_NC = _build_nc()
    return _CACHED_NC


def _prep_in_maps(representation, W1, b1, W2, b2):
    rep = np.asarray(representation, dtype=np.float32).reshape(B, R)
    w1 = np.asarray(W1, dtype=np.float32)
    w2 = np.asarray(W2, dtype=np.float32)
    b1 = np.asarray(b1, dtype=np.float32)
    b2 = np.asarray(b2, dtype=np.float32)

    w1p = np.ascontiguousarray(
        w1.T.reshape(RC, 128, HID).transpose(1, 0, 2).reshape(128, RC * HID)
    )
    w2p = np.ascontiguousarray(
        w2.T.reshape(HC, 128, OUT).transpose(1, 0, 2).reshape(128, HC * OUT)
    )
    prow = np.zeros((1, PROWW), dtype=np.float32)
    prow[0, B1OFF : B1OFF + HID] = b1
    prow[0, ONOFF : ONOFF + 128] = 1.0
    prow[0, B2OFF : B2OFF + OUT] = b2

    in_maps = []
    for c in range(N_CORES):
        xt = rep[c * BPC : (c + 1) * BPC].T  # [R, BPC]
        pk1a = np.zeros((128, PK1AW), dtype=np.float32)
        pk1a[:, XTOFF : XTOFF + RC * BPC] = (
            xt.reshape(RC, 128, BPC).transpose(1, 0, 2).reshape(128, RC * BPC)
        )
        pk1a[0:BPC, I4OFF : I4OFF + BPC] = np.eye(BPC, dtype=np.float32)
        for b in range(BPC):
            pk1a[b, SELOFF + b * 128 : SELOFF + (b + 1) * 128] = 1.0
        in_maps.append({"pk1a": pk1a, "prow": prow, "w1": w1p, "w2": w2p})
    return in_maps


def run_sharded(representation, W1, b1, W2, b2, **run_kwargs):
    """Compile+run on 8 cores; returns (full_output, BassKernelResults)."""
    from concourse.bass_utils import run_bass_kernel_spmd

    nc = _get_nc()
    in_maps = _prep_in_maps(representation, W1, b1, W2, b2)
    res = run_bass_kernel_spmd(nc, in_maps, core_ids=list(range(N_CORES)), **run_kwargs)
    full = np.concatenate([r["out"] for r in res.results], axis=0)
    return full, res


def kernel(representation, size_matrix=None, W1=None, b1=None, W2=None, b2=None):
    # size_matrix only contributes its shape in the reference (ones_like);
    # its values are unused.
    full, _ = run_sharded(representation, W1, b1, W2, b2)
    return full



# revision 21
# speedup vs baseline: 1.3298x; 1.3298x over previous
"""Trainium2 Bass kernel for nn_Decoder_14894946583396 (dense_mlp).

Reference computation:
    sized = broadcast(representation[B,1,R] -> [B,S,R])   (ones @ rep)
    h     = relu(sized @ W1^T + b1)                       [B,S,HID]
    out   = h @ W2^T + b2                                 [B,S,OUT]

Every position s within batch b receives the identical input row, so
    row[b] = relu(rep[b] @ W1^T + b1) @ W2^T + b2         [B,OUT]
    out[b, s, :] = row[b]  for all s

Data-parallel across 8 NeuronCores: 4 batches per core, replicated
weights.  The per-core kernel is organized so DMA queues are never idle:

  1. Tiny HWDGE inputs land first (x^T on SP ring; b1/b2/selectors on
     the ACT ring) while W1 streams on the SWDGE queue in chunks.
  2. A few warmup matmuls raise the PE HAM clock (1.2 -> 2.4 GHz)
     under the input DMAs.
  3. L1 is W1-stationary: per (rc,hc) chunk, lhsT = W1 block [128,128]
     (bf16, FWL), rhs = x^T block [128,4]; accumulates H^T [128h, 4m]
     directly in PSUM -- no transposes.  Pipelined per W1 chunk.
  4. relu+b1 fused in the PSUM->SBUF activation (per-partition bias AP).
  5. L2 pipelined per W2 chunk: lhsT = H^T block [128,4] stationary,
     rhs = W2 block [128,512]; Y [4,512] x2 in PSUM.  b2 added by DVE
     during the PSUM->SBUF move using a host-pre-broadcast b2 tile.
  6. Per batch: a K=4 selector matmul replicates row[b] across all 128
     partitions; one engine (DVE/ACT alternating) moves both halves to
     SBUF; a single 4 MiB DMA per batch writes all S rows using a
     0-stride (broadcast) source AP -- no replication copies.
"""

import sys

import numpy as np

if "/opt/trn_rl_repo" not in sys.path:
    sys.path.insert(0, "/opt/trn_rl_repo")

B, S, R = 32, 1024, 1024
HID, OUT = 512, 1024
N_CORES = 8
BPC = B // N_CORES  # batches per core

RC = R // 128  # layer-1 contraction chunks
HC = HID // 128  # layer-2 contraction chunks
OC = OUT // 512  # 512-wide output column chunks
W1_CHUNKS = 1  # W1 DMA chunks (2 rc each)
W2_CHUNKS = 1  # W2 DMA chunks (1 hc each)
N_WARMUP = 6

N_COPIES = S // 128  # broadcast factor per output DMA (0-stride AP)

_CACHED_NC = None


def _build_nc():
    import concourse.bass as bass
    import concourse.mybir as mybir
    from concourse.tile import TileContext, add_dep_helper

    f32 = mybir.dt.float32
    bf16 = mybir.dt.bfloat16
    relu = mybir.ActivationFunctionType.Relu
    fcopy = mybir.ActivationFunctionType.Copy
    nc = bass.Bass()

    # xt: x^T chunks (32 cols) + a ones row (4 cols, partition 0)
    xt = nc.dram_tensor("xt", [128, RC * BPC + BPC], bf16, kind="ExternalInput")
    # aux row: b1 (512) | b2 (1024) on partition 0
    aux = nc.dram_tensor("aux", [1, HID + OUT], bf16, kind="ExternalInput")
    selt = nc.dram_tensor("selt", [BPC, BPC * 128], f32, kind="ExternalInput")
    w1 = nc.dram_tensor("w1", [128, RC * HID], bf16, kind="ExternalInput")
    w2 = nc.dram_tensor("w2", [128, HC * OUT], bf16, kind="ExternalInput")
    out = nc.dram_tensor("out", [BPC, S, OUT], f32, kind="ExternalOutput")
    import os

    debug = bool(os.environ.get("KERNEL_DEBUG"))
    if debug:
        dbg_ht = nc.dram_tensor("dbg_ht", [128, HC * BPC], f32, kind="ExternalOutput")
        dbg_y = nc.dram_tensor("dbg_y", [BPC, OUT], f32, kind="ExternalOutput")

    with TileContext(nc) as tc:
        with (
            tc.tile_pool(name="const", bufs=1) as cpool,
            tc.tile_pool(name="psum_w", bufs=1, space="PSUM") as pp_w,
            tc.tile_pool(name="psum_h", bufs=1, space="PSUM") as pp_h,
            tc.tile_pool(name="psum_y", bufs=1, space="PSUM") as pp_y,
            tc.tile_pool(name="psum_bc", bufs=2, space="PSUM") as pp_bc,
        ):
            # ---- input DMAs ------------------------------------------------
            xt_sb = cpool.tile([128, RC * BPC + BPC], bf16, tag="xt")
            dma_xt = nc.sync.dma_start(out=xt_sb[:, :], in_=xt[:, :])

            aux_sb = cpool.tile([1, HID + OUT], bf16, tag="aux")
            dma_aux = nc.scalar.dma_start(out=aux_sb[0:1, :], in_=aux[0:1, :])
            sel_sb = cpool.tile([BPC, BPC * 128], f32, tag="sel")
            dma_sel = nc.scalar.dma_start(out=sel_sb[:, :], in_=selt[:, :])

            w1_sb = cpool.tile([128, RC * HID], bf16, tag="w1")
            w1_dmas = []
            w1_cols = RC * HID // W1_CHUNKS
            for c in range(W1_CHUNKS):
                d = nc.gpsimd.dma_start(
                    out=w1_sb[:, c * w1_cols : (c + 1) * w1_cols],
                    in_=w1[:, c * w1_cols : (c + 1) * w1_cols],
                )
                w1_dmas.append(d)
            w2_sb = cpool.tile([128, HC * OUT], bf16, tag="w2")
            w2_dmas = []
            w2_cols = HC * OUT // W2_CHUNKS
            for c in range(W2_CHUNKS):
                d = nc.gpsimd.dma_start(
                    out=w2_sb[:, c * w2_cols : (c + 1) * w2_cols],
                    in_=w2[:, c * w2_cols : (c + 1) * w2_cols],
                )
                w2_dmas.append(d)

            # ---- PE warmup on zeros (under the input DMAs) -----------------
            wm_sb = cpool.tile([128, 512], f32, tag="wm")
            nc.vector.memset(wm_sb[:, :], 0.0)
            pw = pp_w.tile([128, 512], f32, tag="w")
            for k in range(N_WARMUP):
                wmm = nc.tensor.matmul(
                    pw[:, :],
                    lhsT=wm_sb[:, 0:128],
                    rhs=wm_sb[:, :],
                    start=True,
                    stop=True,
                )
            # pre-observe the xt lane so L1's first matmul needs only its
            # w1-chunk wait
            add_dep_helper(wmm.ins, dma_xt.ins, sync=True, reason="observe xt")

            # ---- L1: H^T[h, m] = W1 @ x, relu(+b1) -------------------------
            # rc-major so each W1 chunk is consumed as it lands; 4 hc
            # accumulation groups interleave in one PSUM tile.
            ph = pp_h.tile([128, HC * BPC], f32, tag="h")
            for hc in range(HC):
                for rc in range(RC):
                    nc.tensor.matmul(
                        ph[:, hc * BPC : (hc + 1) * BPC],
                        lhsT=w1_sb[:, rc * HID + hc * 128 : rc * HID + (hc + 1) * 128],
                        rhs=xt_sb[:, rc * BPC : (rc + 1) * BPC],
                        start=(rc == 0),
                        stop=False,
                        skip_group_check=True,
                    )
                # b1 as a K=1 ones-matmul: ph[h, m] += b1[h] * 1
                nc.tensor.matmul(
                    ph[:, hc * BPC : (hc + 1) * BPC],
                    lhsT=aux_sb[0:1, hc * 128 : (hc + 1) * 128],
                    rhs=xt_sb[0:1, RC * BPC : RC * BPC + BPC],
                    start=False,
                    stop=True,
                    skip_group_check=True,
                )
            # throwaway matmul with a free wait slot pre-observes w2 chunk 0's
            # lane, so L2's first matmul only needs the ACT (relu) wait
            obs = nc.tensor.matmul(
                pw[:, 0:BPC],
                lhsT=wm_sb[:, 0:128],
                rhs=wm_sb[:, 0:BPC],
                start=True,
                stop=True,
            )
            add_dep_helper(obs.ins, w2_dmas[0].ins, sync=True, reason="observe w2c0")
            ht_sb = cpool.tile([128, HC * BPC], bf16, tag="ht")
            for hc in range(HC):
                nc.scalar.activation(
                    ht_sb[:, hc * BPC : (hc + 1) * BPC],
                    ph[:, hc * BPC : (hc + 1) * BPC],
                    relu,
                )

            # ---- L2: Y[m, o] = H @ W2^T (+b2 on the way out) ---------------
            py = []
            for oc in range(OC):
                py_oc = pp_y.tile([BPC, 512], f32, tag=f"y{oc}", name=f"py{oc}")
                py.append(py_oc)
            for oc in range(OC):
                for hc in range(HC):
                    nc.tensor.matmul(
                        py[oc][:, :],
                        lhsT=ht_sb[:, hc * BPC : (hc + 1) * BPC],
                        rhs=w2_sb[:, hc * OUT + oc * 512 : hc * OUT + oc * 512 + 512],
                        start=(hc == 0),
                        stop=False,
                        skip_group_check=True,
                    )
                # b2 as a K=1 ones-matmul: py[m, o] += 1 * b2[o]
                bmm = nc.tensor.matmul(
                    py[oc][:, :],
                    lhsT=xt_sb[0:1, RC * BPC : RC * BPC + BPC],
                    rhs=aux_sb[0:1, HID + oc * 512 : HID + (oc + 1) * 512],
                    start=False,
                    stop=True,
                    skip_group_check=True,
                )
            # free wait slot: pre-observe the selector lane for the broadcast
            add_dep_helper(bmm.ins, dma_sel.ins, sync=True, reason="observe sel")
            y_sb = cpool.tile([BPC, OUT], f32, tag="y")
            for oc in range(OC):
                nc.vector.tensor_copy(y_sb[:, oc * 512 : (oc + 1) * 512], py[oc][:, :])
            if debug:
                ht_f32 = cpool.tile([128, HC * BPC], f32, tag="htf")
                cpy = nc.vector.tensor_copy(ht_f32[:, :], ht_sb[:, :])
                nc.gpsimd.dma_start(out=dbg_ht[:, :], in_=ht_f32[:, :])
                nc.gpsimd.dma_start(out=dbg_y[:, :], in_=y_sb[:, :])

            # ---- broadcast rows across partitions, store -------------------
            out_dmas = []
            last_act = None
            last_dve = None
            for b in range(BPC):
                yt = cpool.tile([128, OUT], f32, tag=f"yt{b}")
                copy_eng = "dve" if b % 2 == 0 else "act"
                for oc in range(OC):
                    pb = pp_bc.tile([128, 512], f32, tag="bc")
                    mm = nc.tensor.matmul(
                        pb[:, :],
                        lhsT=sel_sb[0:BPC, b * 128 : (b + 1) * 128],
                        rhs=y_sb[0:BPC, oc * 512 : (oc + 1) * 512],
                        start=True,
                        stop=True,
                    )
                    last_mm = mm
                    dst = yt[:, oc * 512 : (oc + 1) * 512]
                    if copy_eng == "dve":
                        last_dve = nc.vector.tensor_copy(dst, pb[:, :])
                    else:
                        last_act = nc.scalar.activation(dst, pb[:, :], fcopy)
                # single DMA writes all S rows of batch b via a 0-stride
                # broadcast source AP (each partition's 4 KiB row is read
                # N_COPIES times)
                d = nc.sync.dma_start(
                    out=out[b, :, :].rearrange("(p c) o -> p c o", c=N_COPIES),
                    in_=yt[:, :]
                    .rearrange("p (c o) -> p c o", c=1)
                    .to_broadcast((128, N_COPIES, OUT)),
                )
                out_dmas.append(d)

            # single-sync-wait discipline for the TileContext exit drain:
            # chain SP nops, one dependency each, so SP's vector clock
            # observes every DMA lane / engine tick before the drain.
            tail = (
                out_dmas
                + w1_dmas
                + w2_dmas
                + [dma_xt, dma_sel, dma_aux, last_mm, last_act, last_dve]
            )
            tail = [t for t in tail if t is not None]
            for d in tail:
                tn = nc.sync.nop(nofuse=True)
                add_dep_helper(
                    tn.ins, d.ins, sync=True, reason="observe final ticks pre-drain"
                )

    return nc


def _get_nc():
    global _CACHED_NC
    if _CACHED_NC is None:
        _CACHED_NC = _build_nc()
    return _CACHED_NC


def _prep_in_maps(representation, W1, b1, W2, b2):
    import ml_dtypes

    bf16 = ml_dtypes.bfloat16

    rep = np.asarray(representation, dtype=np.float32).reshape(B, R)
    w1 = np.asarray(W1, dtype=np.float32)
    w2 = np.asarray(W2, dtype=np.float32)
    b1 = np.asarray(b1, dtype=np.float32)
    b2 = np.asarray(b2, dtype=np.float32)

    # w1p[p, rc*HID + hc*128 + j] = W1[hc*128+j, rc*128+p]
    w1p = np.ascontiguousarray(
        w1.reshape(HC, 128, RC, 128).transpose(3, 2, 0, 1).reshape(128, RC * HID)
    ).astype(bf16)
    # w2p[p, hc*OUT + oc*512 + o] = W2[oc*512+o, hc*128+p]
    w2p = np.ascontiguousarray(
        w2.reshape(OC, 512, HC, 128).transpose(3, 2, 0, 1).reshape(128, HC * OUT)
    ).astype(bf16)
    # aux row: b1 | b2 (bf16)
    auxp = np.zeros((1, HID + OUT), dtype=np.float32)
    auxp[0, 0:HID] = b1
    auxp[0, HID : HID + OUT] = b2
    auxp = auxp.astype(bf16)
    # selectors: sel[k, b*128 + i] = (k == b)
    selp = np.zeros((BPC, BPC * 128), dtype=np.float32)
    for b in range(BPC):
        selp[b, b * 128 : (b + 1) * 128] = 1.0

    in_maps = []
    for c in range(N_CORES):
        xtc = rep[c * BPC : (c + 1) * BPC].T  # [R, BPC]
        # xt[p, rc*BPC + m] = rep[m, rc*128+p]; then a ones row (partition 0)
        xtp = np.zeros((128, RC * BPC + BPC), dtype=np.float32)
        xtp[:, 0 : RC * BPC] = (
            xtc.reshape(RC, 128, BPC).transpose(1, 0, 2).reshape(128, RC * BPC)
        )
        xtp[0, RC * BPC : RC * BPC + BPC] = 1.0
        in_maps.append(
            {
                "xt": xtp.astype(bf16),
                "aux": auxp,
                "selt": selp,
                "w1": w1p,
                "w2": w2p,
            }
        )
    return in_maps


def run_sharded(representation, W1, b1, W2, b2, **run_kwargs):
    """Compile+run on 8 cores; returns (full_output, BassKernelResults)."""
    from concourse.bass_utils import run_bass_kernel_spmd

    nc = _get_nc()
    in_maps = _prep_in_maps(representation, W1, b1, W2, b2)
    res = run_bass_kernel_spmd(nc, in_maps, core_ids=list(range(N_CORES)), **run_kwargs)
    full = np.concatenate([r["out"] for r in res.results], axis=0)
    return full, res


def kernel(representation, size_matrix=None, W1=None, b1=None, W2=None, b2=None):
    # size_matrix only contributes its shape in the reference (ones_like);
    # its values are unused.
    full, _ = run_sharded(representation, W1, b1, W2, b2)
    return full


# revision 25
# speedup vs baseline: 1.3403x; 1.0079x over previous
"""Trainium2 Bass kernel for nn_Decoder_14894946583396 (dense_mlp).

Reference computation:
    sized = broadcast(representation[B,1,R] -> [B,S,R])   (ones @ rep)
    h     = relu(sized @ W1^T + b1)                       [B,S,HID]
    out   = h @ W2^T + b2                                 [B,S,OUT]

Every position s within batch b receives the identical input row, so
    row[b] = relu(rep[b] @ W1^T + b1) @ W2^T + b2         [B,OUT]
    out[b, s, :] = row[b]  for all s

Data-parallel across 8 NeuronCores: 4 batches per core, replicated
weights.  The per-core kernel keeps the DMA queues busy end to end:

  1. Tiny HWDGE inputs (x^T on the SP ring; b1 / selectors / b2-row on
     the ACT ring) land while W1 streams on the SWDGE queue in chunks.
  2. A short warmup raises the PE HAM clock under the input DMAs.
  3. L1 is W1-stationary: per (rc,hc), lhsT = W1 block [128,128] (bf16,
     FWL), rhs = x^T block [128,4]; accumulates H^T [128h, 4m] directly
     -- no transposes.  rc-major so each W1 chunk is consumed as it
     lands; each hc accumulation group lives in its own PSUM bank
     (interleaving groups inside ONE bank corrupts accumulation).
  4. b1 enters as a K=1 ones-matmul; relu on ACT casting to bf16.
  5. L2 pipelined per oc-major W2 chunk: lhsT = H^T block [128,4]
     stationary, rhs = W2 block [128,512]; Y [4,512] per oc in PSUM,
     groups sequential per bank.
  6. Per batch: a K=5 selector matmul replicates row[b] across all 128
     partitions AND adds b2 (selector row 4 = ones, y_sb row 4 = b2);
     one engine (DVE/ACT alternating per batch) moves both halves to
     SBUF; a single 4 MiB DMA per batch writes all S rows using a
     0-stride (broadcast) source AP -- no replication copies.

Single-sync-wait discipline: a BIR Matmult carries waits for BOTH of
its operands, and any instruction may carry at most ONE semaphore
wait.  Artificial add_dep_helper edges pre-observe upcoming DMA lanes
on instructions with a free wait slot (warmup observes xt; bias
matmuls observe sel/b2 lanes; a throwaway matmul observes w2c0), and
both PSUM->SBUF copies of a batch stay on one engine so its output DMA
needs a single wait.  A chain of 1-wait SP nops pre-observes every
final tick for the TileContext exit drain.
"""

import sys

import numpy as np

if "/opt/trn_rl_repo" not in sys.path:
    sys.path.insert(0, "/opt/trn_rl_repo")

B, S, R = 32, 1024, 1024
HID, OUT = 512, 1024
N_CORES = 8
BPC = B // N_CORES  # batches per core

RC = R // 128  # layer-1 contraction chunks
HC = HID // 128  # layer-2 contraction chunks
OC = OUT // 512  # 512-wide output column chunks
W1_CHUNKS = 4  # W1 DMA chunks (2 rc each)
W2_CHUNKS = 4  # W2 DMA chunks (oc-major, 2 hc each)
N_WARMUP = 4

N_COPIES = S // 128  # broadcast factor per output DMA (0-stride AP)

_CACHED_NC = None


def _build_nc():
    import concourse.bass as bass
    import concourse.mybir as mybir
    from concourse.tile import TileContext, add_dep_helper

    f32 = mybir.dt.float32
    bf16 = mybir.dt.bfloat16
    relu = mybir.ActivationFunctionType.Relu
    fcopy = mybir.ActivationFunctionType.Copy
    nc = bass.Bass()

    # xt: x^T chunks (32 cols) + a ones row (4 cols, partition 0)
    xt = nc.dram_tensor("xt", [128, RC * BPC + BPC], bf16, kind="ExternalInput")
    # aux row: b1 on partition 0
    aux = nc.dram_tensor("aux", [1, HID], bf16, kind="ExternalInput")
    # selectors (+ row 4 = ones, the b2 gate)
    selt = nc.dram_tensor("selt", [BPC + 1, BPC * 128], f32, kind="ExternalInput")
    # b2 row, f32 (lands directly in y_sb row 4)
    b2f = nc.dram_tensor("b2f", [1, OUT], f32, kind="ExternalInput")
    w1 = nc.dram_tensor("w1", [128, RC * HID], bf16, kind="ExternalInput")
    w2 = nc.dram_tensor("w2", [128, HC * OUT], bf16, kind="ExternalInput")
    out = nc.dram_tensor("out", [BPC, S, OUT], f32, kind="ExternalOutput")

    with TileContext(nc) as tc:
        with (
            tc.tile_pool(name="const", bufs=1) as cpool,
            tc.tile_pool(name="psum_h", bufs=1, space="PSUM") as pp_h,
            tc.tile_pool(name="psum_y", bufs=1, space="PSUM") as pp_y,
            tc.tile_pool(name="psum_bc", bufs=2, space="PSUM") as pp_bc,
        ):
            # ---- input DMAs ------------------------------------------------
            xt_sb = cpool.tile([128, RC * BPC + BPC], bf16, tag="xt")
            dma_xt = nc.sync.dma_start(out=xt_sb[:, :], in_=xt[:, :])

            aux_sb = cpool.tile([1, HID], bf16, tag="aux")
            dma_aux = nc.scalar.dma_start(out=aux_sb[0:1, :], in_=aux[0:1, :])
            sel_sb = cpool.tile([BPC + 1, BPC * 128], f32, tag="sel")
            dma_sel = nc.scalar.dma_start(out=sel_sb[:, :], in_=selt[:, :])
            # y_sb rows 0-3 = Y (written later by DVE); row 4 = b2 via DMA
            y_sb = cpool.tile([BPC + 1, OUT], f32, tag="y")
            dma_b2 = nc.scalar.dma_start(out=y_sb[BPC : BPC + 1, :], in_=b2f[0:1, :])

            w1_sb = cpool.tile([128, RC * HID], bf16, tag="w1")
            w1_dmas = []
            w1_cols = RC * HID // W1_CHUNKS
            for c in range(W1_CHUNKS):
                d = nc.gpsimd.dma_start(
                    out=w1_sb[:, c * w1_cols : (c + 1) * w1_cols],
                    in_=w1[:, c * w1_cols : (c + 1) * w1_cols],
                )
                w1_dmas.append(d)
            # w2 packed oc-major: [p, oc*HC*512 + hc*512 + o]
            w2_sb = cpool.tile([128, HC * OUT], bf16, tag="w2")
            w2_dmas = []
            w2_cols = HC * OUT // W2_CHUNKS
            for c in range(W2_CHUNKS):
                d = nc.gpsimd.dma_start(
                    out=w2_sb[:, c * w2_cols : (c + 1) * w2_cols],
                    in_=w2[:, c * w2_cols : (c + 1) * w2_cols],
                )
                w2_dmas.append(d)

            # ---- L1: H^T[h, m] = W1 @ x (+b1), relu ------------------------
            # a PE nop pre-observes the xt lane so L1's first matmul needs
            # only its w1-chunk wait
            wn = nc.tensor.nop(nofuse=True)
            add_dep_helper(wn.ins, dma_xt.ins, sync=True, reason="observe xt")
            # rc-major so each W1 chunk is consumed as it lands; one PSUM
            # bank per hc keeps each accumulation group sequential within
            # its bank.
            ph = []
            for hc in range(HC):
                ph_hc = pp_h.tile([128, BPC], f32, tag=f"h{hc}", name=f"ph{hc}")
                ph.append(ph_hc)
            for rc in range(RC):
                for hc in range(HC):
                    nc.tensor.matmul(
                        ph[hc][:, :],
                        lhsT=w1_sb[:, rc * HID + hc * 128 : rc * HID + (hc + 1) * 128],
                        rhs=xt_sb[:, rc * BPC : (rc + 1) * BPC],
                        start=(rc == 0),
                        stop=False,
                        skip_group_check=True,
                    )
            # b1 as a K=1 ones-matmul: ph[h, m] += b1[h] * 1.  The later
            # ones have free wait slots: pre-observe the sel and b2 lanes
            # for the broadcast phase.
            for hc in range(HC):
                bmm = nc.tensor.matmul(
                    ph[hc][:, :],
                    lhsT=aux_sb[0:1, hc * 128 : (hc + 1) * 128],
                    rhs=xt_sb[0:1, RC * BPC : RC * BPC + BPC],
                    start=False,
                    stop=True,
                    skip_group_check=True,
                )
                if hc == 1:
                    add_dep_helper(bmm.ins, dma_sel.ins, sync=True, reason="obs sel")
                if hc == 2:
                    add_dep_helper(bmm.ins, dma_b2.ins, sync=True, reason="obs b2")
                if hc == 3:
                    # pre-observe w2 chunk 0's lane so L2's first matmul
                    # only needs the ACT (relu) wait
                    add_dep_helper(
                        bmm.ins, w2_dmas[0].ins, sync=True, reason="obs w2c0"
                    )
            ht_sb = cpool.tile([128, HC * BPC], bf16, tag="ht")
            for hc in range(HC):
                nc.scalar.activation(
                    ht_sb[:, hc * BPC : (hc + 1) * BPC],
                    ph[hc][:, :],
                    relu,
                )

            # ---- L2: Y[m, o] = H @ W2^T ------------------------------------
            # oc-major to match the oc-major W2 chunks; each oc's group is
            # sequential in its own bank.  The PSUM->SBUF copy of oc=0
            # overlaps oc=1's matmuls.
            py = []
            for oc in range(OC):
                py_oc = pp_y.tile([BPC, 512], f32, tag=f"y{oc}", name=f"py{oc}")
                py.append(py_oc)
            y_copies = []
            for oc in range(OC):
                for hc in range(HC):
                    nc.tensor.matmul(
                        py[oc][:, :],
                        lhsT=ht_sb[:, hc * BPC : (hc + 1) * BPC],
                        rhs=w2_sb[:, oc * HC * 512 + hc * 512 : oc * HC * 512 + (hc + 1) * 512],
                        start=(hc == 0),
                        stop=(hc == HC - 1),
                        skip_group_check=True,
                    )
                c = nc.vector.tensor_copy(y_sb[0:BPC, oc * 512 : (oc + 1) * 512], py[oc][:, :])
                y_copies.append(c)

            # ---- broadcast rows across partitions (+b2), store -------------
            out_dmas = []
            last_act = None
            last_dve = y_copies[-1]
            for b in range(BPC):
                yt = cpool.tile([128, OUT], f32, tag=f"yt{b}")
                copy_eng = "dve" if b % 2 == 0 else "act"
                for oc in range(OC):
                    pb = pp_bc.tile([128, 512], f32, tag="bc", name=f"pb{b}_{oc}")
                    mm = nc.tensor.matmul(
                        pb[:, :],
                        lhsT=sel_sb[0 : BPC + 1, b * 128 : (b + 1) * 128],
                        rhs=y_sb[0 : BPC + 1, oc * 512 : (oc + 1) * 512],
                        start=True,
                        stop=True,
                    )
                    last_mm = mm
                    dst = yt[:, oc * 512 : (oc + 1) * 512]
                    if copy_eng == "dve":
                        last_dve = nc.vector.tensor_copy(dst, pb[:, :])
                    else:
                        last_act = nc.scalar.activation(dst, pb[:, :], fcopy)
                # single DMA writes all S rows of batch b via a 0-stride
                # broadcast source AP (each partition's 4 KiB row is read
                # N_COPIES times)
                d = nc.sync.dma_start(
                    out=out[b, :, :].rearrange("(p c) o -> p c o", c=N_COPIES),
                    in_=yt[:, :]
                    .rearrange("p (c o) -> p c o", c=1)
                    .to_broadcast((128, N_COPIES, OUT)),
                )
                out_dmas.append(d)

            # single-sync-wait discipline for the TileContext exit drain:
            # chain SP nops, one dependency each, so SP's vector clock
            # observes every DMA lane / engine tick before the drain.
            tail = (
                out_dmas
                + w1_dmas
                + w2_dmas
                + [dma_xt, dma_sel, dma_aux, dma_b2, last_mm, last_act, last_dve]
            )
            tail = [t for t in tail if t is not None]
            for d in tail:
                tn = nc.sync.nop(nofuse=True)
                add_dep_helper(
                    tn.ins, d.ins, sync=True, reason="observe final ticks pre-drain"
                )

    return nc


def _get_nc():
    global _CACHED_NC
    if _CACHED_NC is None:
        _CACHED_NC = _build_nc()
    return _CACHED_NC


def _prep_in_maps(representation, W1, b1, W2, b2):
    import ml_dtypes

    bf16 = ml_dtypes.bfloat16

    rep = np.asarray(representation, dtype=np.float32).reshape(B, R)
    w1 = np.asarray(W1, dtype=np.float32)
    w2 = np.asarray(W2, dtype=np.float32)
    b1 = np.asarray(b1, dtype=np.float32)
    b2 = np.asarray(b2, dtype=np.float32)

    # w1p[p, rc*HID + hc*128 + j] = W1[hc*128+j, rc*128+p]
    w1p = np.ascontiguousarray(
        w1.reshape(HC, 128, RC, 128).transpose(3, 2, 0, 1).reshape(128, RC * HID)
    ).astype(bf16)
    # w2p[p, oc*HC*512 + hc*512 + o] = W2[oc*512+o, hc*128+p]  (oc-major)
    w2p = np.ascontiguousarray(
        w2.reshape(OC, 512, HC, 128).transpose(3, 0, 2, 1).reshape(128, HC * OUT)
    ).astype(bf16)
    # aux row: b1 (bf16)
    auxp = b1.reshape(1, HID).astype(bf16)
    # selectors: sel[k, b*128 + i] = (k == b); row BPC = ones (b2 gate)
    selp = np.zeros((BPC + 1, BPC * 128), dtype=np.float32)
    for b in range(BPC):
        selp[b, b * 128 : (b + 1) * 128] = 1.0
    selp[BPC, :] = 1.0
    b2p = b2.reshape(1, OUT).copy()

    in_maps = []
    for c in range(N_CORES):
        xtc = rep[c * BPC : (c + 1) * BPC].T  # [R, BPC]
        # xt[p, rc*BPC + m] = rep[m, rc*128+p]; then a ones row (partition 0)
        xtp = np.zeros((128, RC * BPC + BPC), dtype=np.float32)
        xtp[:, 0 : RC * BPC] = (
            xtc.reshape(RC, 128, BPC).transpose(1, 0, 2).reshape(128, RC * BPC)
        )
        xtp[0, RC * BPC : RC * BPC + BPC] = 1.0
        in_maps.append(
            {
                "xt": xtp.astype(bf16),
                "aux": auxp,
                "selt": selp,
                "b2f": b2p,
                "w1": w1p,
                "w2": w2p,
            }
        )
    return in_maps


def run_sharded(representation, W1, b1, W2, b2, **run_kwargs):
    """Compile+run on 8 cores; returns (full_output, BassKernelResults)."""
    from concourse.bass_utils import run_bass_kernel_spmd

    nc = _get_nc()
    in_maps = _prep_in_maps(representation, W1, b1, W2, b2)
    res = run_bass_kernel_spmd(nc, in_maps, core_ids=list(range(N_CORES)), **run_kwargs)
    full = np.concatenate([r["out"] for r in res.results], axis=0)
    return full, res


def kernel(representation, size_matrix=None, W1=None, b1=None, W2=None, b2=None):
    # size_matrix only contributes its shape in the reference (ones_like);
    # its values are unused.
    full, _ = run_sharded(representation, W1, b1, W2, b2)
    return full
